# revision 8
# baseline (speedup 1.0000x reference)
"""DechirpSTFT Trainium2 kernel (8 NeuronCores).

Math: out[d,b,w,:] = FFT_1024(chirp * resample_d(hann * window(x[b], w)))

Factorization per (d, b):
  - window + hann + linear-interp resample  ->  banded matrix G_d applied by
    TensorE directly to x held in SBUF as [128, 4096] (window = stride-4
    column slice; hop 512 = 4 cols of 128).  G's columns emit y in radix-2
    DIT order: z-tile (n2, t) holds y[2*(128 t + p) + n2].
  - radix-2 DIT FFT (1024 = 512 x 2): stage-1 = two 512-point complex DFT
    matrices M_{n2}[n1,k1] = chirp[2 n1 + n2] * W512^{n1 k1} * W1024^{n2 k1}
    (chirp + twiddle folded in), applied by TensorE (contraction over n1).
  - tail: Y[k1] = V0 + V1, Y[k1+512] = V0 - V1 on VectorE (doubles as the
    PSUM evacuation), written re/im-interleaved for a contiguous output DMA.
All matmuls in float32r (1 cyc/row at N>=256, ~1.5e-4 rel err).
Each core owns 2 of the 16 chirp rates.
"""

import numpy as np

K = 1024
HOP = 512
CHIRP_A = 0.5
NB = 2
NX = 524288
W = (NX - K) // HOP + 1          # 1023
D = 16
NCORES = 8
DLOC = D // NCORES               # 2 chirp rates per core
K1, K2 = 512, 2
WT = 256                          # windows per chunk (matmul moving dim)
NWC = 4                           # ceil(1023/256)
XCOLS = 4104                      # 4096 cols + pad so window 1023 reads zeros

_NC_CACHE = {}
_LAST_RESULTS = {}


def _host_tables_all(dlnf):
    """(16,) -> lo (D,K) int32, frac (D,K) f32.  Computed with jax on CPU,
    bit-exactly mirroring reference.py's fp32 pipeline (numpy's fp32
    exp/log1p differ from XLA's by enough to shift idx by ~1e-3 samples)."""
    import jax
    import jax.numpy as jnp

    cpu = jax.devices("cpu")[0]
    with jax.default_device(cpu):
        betas = 2.0 * jnp.asarray(np.asarray(dlnf, dtype=np.float32))
        safe = jnp.abs(betas) < 1e-8
        bs = jnp.where(safe, jnp.float32(1e-8), betas)
        tau = jnp.linspace(0.0, 1.0, K, dtype=jnp.float32)
        t_src = 2.0 / bs[:, None] * jnp.log1p(
            tau[None, :] * (jnp.exp(bs)[:, None] - 1.0)) - 1.0
        identity = jnp.linspace(-1.0, 1.0, K, dtype=jnp.float32)
        t_src = jnp.where(safe[:, None], identity[None, :], t_src)
        idx = (t_src + 1.0) * 0.5 * (K - 1)
        lo = jnp.clip(idx.astype(jnp.int32), 0, K - 2)
        frac = idx - lo.astype(idx.dtype)
    return np.asarray(lo), np.asarray(frac).astype(np.float32)


def _jt_slots(t):
    """Source x j-tile (of 128) program slots for z-tile t (same for all d)."""
    return [2 * t - 1, 2 * t, 2 * t + 1, 2 * t + 2]


def _build_g(lo_pair, frac_pair):
    """Interp stationaries, packed [128, 2*2*4*4*128] fp32.
    Col block ((d2*2+n2)*4+t)*4+s holds G[q, p]: src j=128*jt+q -> n=256t+2p+n2."""
    hann = (0.5 * (1.0 - np.cos(2.0 * np.pi * np.arange(K) / K))).astype(np.float32)
    g = np.zeros((128, DLOC * 2 * 4 * 4 * 128), dtype=np.float32)
    nn = np.arange(K)
    n2a, nh = nn & 1, nn >> 1
    ta, pa = nh >> 7, nh & 127
    for d2 in range(DLOC):
        lo = lo_pair[d2]
        frac = frac_pair[d2]
        alpha = ((1.0 - frac) * hann[lo]).astype(np.float32)
        beta = (frac * hann[lo + 1]).astype(np.float32)
        for j, val in ((lo, alpha), (lo + 1, beta)):
            jt, q = j >> 7, j & 127
            s = jt - (2 * ta - 1)
            if not np.all((s >= 0) & (s < 4)):
                raise ValueError("interp band exceeds the 4 source-tile slots")
            flat = ((d2 * 2 + n2a) * 4 + ta) * 4 + s
            np.add.at(g, (q, flat * 128 + pa), val)
    return g


def _build_m1():
    """Stage-1 DFT stationaries [128, 2*2*4*4*128] fp32 (d-independent).
    Col block ((n2*2+pl)*4+kt)*4+mc holds M[q, c]: n1=128kt+q, k1=128mc+c."""
    t_norm = np.linspace(-1.0, 1.0, K).astype(np.float64)
    chirp = np.exp(-1j * CHIRP_A * t_norm ** 2)
    m1 = np.zeros((128, 2 * 2 * 4 * 4 * 128), dtype=np.float32)
    n1g = np.arange(K1)
    k1g = np.arange(K1)
    for n2 in range(2):
        M = (chirp[2 * n1g + n2][:, None]
             * np.exp(-2j * np.pi * np.outer(n1g, k1g) / K1)
             * np.exp(-2j * np.pi * n2 * k1g / K)[None, :])
        for pl in range(2):
            plane = (M.real if pl == 0 else M.imag).astype(np.float32)
            for kt in range(4):
                for mc in range(4):
                    flat = ((n2 * 2 + pl) * 4 + kt) * 4 + mc
                    m1[:, flat * 128:(flat + 1) * 128] = \
                        plane[128 * kt:128 * kt + 128, 128 * mc:128 * mc + 128]
    return m1


def _build_program():
    import concourse.bacc as bacc
    import concourse.mybir as mybir
    from concourse.tile import TileContext

    f32 = mybir.dt.float32
    f32r = mybir.dt.float32r

    nc = bacc.Bacc("TRN2", target_bir_lowering=False, debug=False,
                   num_devices=NCORES)
    xT = nc.dram_tensor("xT", [NB, 128, XCOLS], f32r, kind="ExternalInput")
    g = nc.dram_tensor("g", [128, DLOC * 2 * 4 * 4 * 128], f32r,
                       kind="ExternalInput")
    m1 = nc.dram_tensor("m1", [128, 2 * 2 * 4 * 4 * 128], f32r,
                        kind="ExternalInput")
    out_t = nc.dram_tensor("out", [DLOC, NB, K, W, 2], f32,
                           kind="ExternalOutput")

    def gcol(d2, n2, t, s):
        flat = ((d2 * 2 + n2) * 4 + t) * 4 + s
        return slice(flat * 128, (flat + 1) * 128)

    def m1col(n2, pl, kt, mc):
        flat = ((n2 * 2 + pl) * 4 + kt) * 4 + mc
        return slice(flat * 128, (flat + 1) * 128)

    with TileContext(nc) as tc:
        with (
            tc.tile_pool(name="resident", bufs=1) as rp,
            tc.tile_pool(name="ysb", bufs=8) as yp,
            tc.tile_pool(name="osb", bufs=4) as op,
            tc.tile_pool(name="py", bufs=4, space="PSUM") as pyp,
            tc.tile_pool(name="pv", bufs=4, space="PSUM") as pvp,
        ):
            xt_sb = []
            for b in range(NB):
                xb = rp.tile([128, XCOLS], f32r, tag=f"x{b}")
                nc.sync.dma_start(out=xb[:, :], in_=xT[b, :, :])
                xt_sb.append(xb)
            g_sb = rp.tile([128, DLOC * 2 * 4 * 4 * 128], f32r, tag="g")
            nc.sync.dma_start(out=g_sb[:, :], in_=g[:, :])
            m1_sb = rp.tile([128, 2 * 2 * 4 * 4 * 128], f32r, tag="m1")
            nc.sync.dma_start(out=m1_sb[:, :], in_=m1[:, :])

            for d2 in range(DLOC):
                for b in range(NB):
                    for wc in range(NWC):
                        w0 = WT * wc
                        wn = min(WT, W - w0)          # 256 or 255 (DMA only)
                        # ---- interp/gather: y in DIT order --------------
                        ytiles = []
                        for t in range(4):
                            py = pyp.tile([128, 2 * WT], f32, tag="py")
                            for n2 in range(2):
                                dst = py[:, n2 * WT:(n2 + 1) * WT]
                                slots = _jt_slots(t)
                                for s, jt in enumerate(slots):
                                    jtc = min(max(jt, 0), 7)
                                    base = 4 * w0 + jtc
                                    rhs = xt_sb[b][:, base:base + 4 * WT:4]
                                    nc.tensor.matmul(
                                        dst, g_sb[:, gcol(d2, n2, t, s)], rhs,
                                        start=(s == 0), stop=(s == 3))
                            ysb = yp.tile([128, 2 * WT], f32r, tag="y")
                            nc.scalar.copy(ysb[:, :], py[:, :])
                            ytiles.append(ysb)
                        # ---- stage-1 DFT + radix-2 tail ------------------
                        for mc in range(4):
                            pv = []
                            v0s = []
                            for pl in range(2):
                                pvt = pvp.tile([128, 2 * WT], f32, tag="pv")
                                for n2 in range(2):
                                    dst = pvt[:, n2 * WT:(n2 + 1) * WT]
                                    for kt in range(4):
                                        nc.tensor.matmul(
                                            dst,
                                            m1_sb[:, m1col(n2, pl, kt, mc)],
                                            ytiles[kt][:, n2 * WT:(n2 + 1) * WT],
                                            start=(kt == 0), stop=(kt == 3))
                                pv.append(pvt)
                                # DVE ops may read only one PSUM operand:
                                # stage V0 (the n2=0 half) through SBUF.
                                v0t = yp.tile([128, WT], f32, tag="v0")
                                nc.scalar.copy(v0t[:, :], pvt[:, 0:WT])
                                v0s.append(v0t)
                            for k2 in range(2):
                                ot = op.tile([128, 2 * WT], f32, tag="o")
                                for pl in range(2):
                                    dst = ot[:, pl:2 * WT:2]
                                    v0 = v0s[pl][:, :]
                                    v1 = pv[pl][:, WT:2 * WT]
                                    if k2 == 0:
                                        nc.vector.tensor_add(dst, v0, v1)
                                    else:
                                        nc.vector.tensor_sub(dst, v0, v1)
                                kb = 128 * mc + 512 * k2
                                nc.sync.dma_start(
                                    out=out_t[d2, b, kb:kb + 128, w0:w0 + wn, :],
                                    in_=ot[:, 0:2 * wn].rearrange(
                                        "p (w r) -> p w r", r=2))
    nc.compile()
    return nc


def _host_prep(x, dlnf):
    x = np.ascontiguousarray(np.asarray(x, dtype=np.float32))
    dlnf = np.asarray(dlnf, dtype=np.float32)
    xT = np.zeros((NB, 128, XCOLS), dtype=np.float32)
    xr = x.reshape(NB, 4096, 128)
    xT[:, :, :4096] = np.transpose(xr, (0, 2, 1))
    m1 = _build_m1()
    lo_all, frac_all = _host_tables_all(dlnf)
    in_maps = []
    for c in range(NCORES):
        gc_ = _build_g(lo_all[DLOC * c: DLOC * (c + 1)],
                       frac_all[DLOC * c: DLOC * (c + 1)])
        in_maps.append({"xT": xT, "g": gc_, "m1": m1})
    return in_maps


def kernel(x, dlnf):
    from concourse.bass_utils import run_bass_kernel_spmd

    in_maps = _host_prep(x, dlnf)
    if "nc" not in _NC_CACHE:
        _NC_CACHE["nc"] = _build_program()
    nc = _NC_CACHE["nc"]
    res = run_bass_kernel_spmd(nc, in_maps, core_ids=list(range(NCORES)))
    _LAST_RESULTS["res"] = res
    outs = []
    for c in range(NCORES):
        o = res.results[c]["out"]                      # [DLOC, NB, K, W, 2] f32
        cplx = (o[..., 0] + 1j * o[..., 1]).astype(np.complex64)
        outs.append(np.transpose(cplx, (0, 1, 3, 2)))  # -> [DLOC, NB, W, K]
    return np.concatenate(outs, axis=0)
